# revision 10
# baseline (speedup 1.0000x reference)
"""FlowNetC correlation kernel for Trainium2 (Bass/Tile), 8-core data-parallel.

Problem: in1, in2: [B=8, C=256, H=96, W=128] fp32
  out[b, o, y, x] = (1/C) * sum_c in1[b,c,y,x] * in2pad[b,c,y+dy,x+dx]
  dy = (o//21 - 10)*2, dx = (o%21 - 10)*2   (441 displacement planes)

Strategy:
  * Data-parallel over batch: one sample per NeuronCore (8 cores).
  * Displacements are even in both axes -> split both images into 4 parity
    classes (y%2, x%2); each class is an independent stride-1 correlation of
    [C, 48, 64] with +-10 neighborhood.
  * Per class, 2D-tile Gram on the PE array: stationary lhsT = 16x8 = 128
    in1 pixels [C-chunk(128) x M=128]; moving rhs = the 36x28 = 1008-pixel
    in2 window around the tile [C-chunk x N], contracted over C in 2
    accumulating matmuls (kc-outer order so consecutive matmuls reuse the
    stationary operand).  PSUM[m, n] then holds, for each in1 pixel m, the
    dot products against every in2 pixel of the window; the 21x21 patch per
    pixel is the 441 correlation outputs for that pixel.
  * bf16 inputs (fp8 fails the 2e-2 gate: measured 3.9e-2), fp32 PSUM.
  * Schedule designed around the PE HAM clock gate (idle >3.4us ->
    re-throttle to 1.2 GHz):
      - 9 warm-up matmuls on a zeroed scratch tile at t=0 bring the PE to
        2.4 GHz while the first input DMA streams, so real matmuls start
        warm with no ramp.
      - b is triple-buffered (rot = cls % 3) and each class's a/b are ONE
        merged DMA each, issued up-front on the gpsimd queue, so classes
        0-2 load immediately and no mid-stream load stall (the baseline's
        8us class-2 gap + 10us HAM re-throttle) can occur.
  * Evacuation: one strided [128, 2, 504] PSUM->SBUF bf16 copy per tile,
    alternating vector/scalar engines (PSUM reads are 1 elem/cycle on both;
    two engines in parallel keep evacuation under the PE time).
  * Stage per ty-row [128, 8, 1008] bf16; 4 u-quarter-trimmed DMAs per
    ty-row (partitions with tile-row u in [4q, 4q+4) need only window rows
    [4q, 4q+24) = 672 of 1008 elements) -> 16.5MB instead of 24.7MB out.
  * Final column shear (v + j2) and layout permutation are done on the
    host (numpy as_strided) which costs no device time.
"""

import numpy as np
from contextlib import ExitStack

import ml_dtypes

import concourse.bass as bass
import concourse.bacc as bacc
import concourse.tile as tile
import concourse.mybir as mybir
from concourse import bass2jax

# ---- problem geometry (hardcoded) ----
B, C, H, W = 8, 256, 96, 128
R = 10                     # class-space displacement radius
GW = 2 * R + 1             # 21
NOUT = GW * GW             # 441
HC, WC = H // 2, W // 2    # 48, 64  class image dims
H2P, W2P = HC + 2 * R, WC + 2 * R   # 68, 84 padded in2 class dims
H1T, W1T = 16, 8           # in1 tile -> M = 128 pixels
H2T, W2T = H1T + 2 * R, W1T + 2 * R  # 36, 28 -> N = 1008
NTY, NTX = HC // H1T, WC // W1T      # 3 x 8 = 24 tiles per class
NTILE = NTY * NTX
NWIN = H2T * W2T           # 1008
KCH = C // 128             # 2 contraction chunks
HTRIM = H2T - H1T // 4 * 3  # 24 dumped window rows per pixel (u-quarter trim)
NTRIM = HTRIM * W2T         # 672 dumped elements per pixel

MM_DT = mybir.dt.bfloat16
MM_NP = ml_dtypes.bfloat16

_CACHE = {}


def _build_nc(mm_dt):
    nc = bacc.Bacc(
        trn_type="TRN2",
        target_bir_lowering=False,
        debug=False,
        num_devices=8,
    )
    # a: in1 pre-tiled on host so each tile's 128 pixels are contiguous
    #    (stationary matmul operand must have a single free dim); kc chunk
    #    as a free dim so the whole class is ONE load.
    # b: x-padded only ([48, 84] per channel) -- the y-pad rows are kept as
    #    persistent zeroed SBUF borders, saving 3.4MB of HBM read traffic.
    a_h = nc.dram_tensor("a", [4, 128, KCH, NTILE, 128], mm_dt,
                         kind="ExternalInput")
    b_h = nc.dram_tensor("b", [4, 128, KCH, HC, W2P], mm_dt,
                         kind="ExternalInput")
    o_h = nc.dram_tensor("o", [4, 128, NTILE, NTRIM],
                         mybir.dt.bfloat16, kind="ExternalOutput")
    a_ap, b_ap, o_ap = a_h.ap(), b_h.ap(), o_h.ap()

    with tile.TileContext(nc) as tc, ExitStack() as ctx:
        # stage gets 6 bufs: early output dumps run while the 14.6MB input
        # stream still owns the DMA engines, so dump completion can lag by
        # several ty-rows; the deep stage ring keeps copies (and through
        # PSUM, the PE) from stalling on it.  a is only double-buffered to
        # pay for it (a is small and its slot frees early enough).
        a_pool = ctx.enter_context(tc.tile_pool(name="a", bufs=2))
        s_pool = ctx.enter_context(tc.tile_pool(name="stage", bufs=6))
        w_pool = ctx.enter_context(tc.tile_pool(name="warm", bufs=1))
        p_pool = ctx.enter_context(tc.tile_pool(name="psum", bufs=3, space="PSUM"))
        pd_pool = ctx.enter_context(tc.tile_pool(name="psd", bufs=1, space="PSUM"))
        ps_dummy = pd_pool.tile([128, 512], mybir.dt.float32)

        # persistent triple-buffered b (rot = cls % 3); the y-pad border
        # rows are zeroed once and never rewritten.  Distinct tags in a
        # bufs=1 pool give each tile its own persistent slot.
        bf_pool = ctx.enter_context(tc.tile_pool(name="bfix", bufs=1))
        b_fix = [bf_pool.tile([128, KCH, H2P, W2P], mm_dt, name=f"bf{r}",
                              tag=f"bf{r}") for r in range(3)]
        # warm-up scratch: memset first on vector so the PE can start
        # spinning immediately.
        wz = w_pool.tile([128, 512], mm_dt, name="wz", tag="wz")
        nc.vector.memset(wz, 0.0)
        # rot0's top border gates the very first matmuls (class-0 ty=0
        # windows include it) -> zero it early on vector; the remaining
        # borders are zeroed from the gpsimd queue interleaved between the
        # load issues below (each is needed several us later).
        nc.vector.memset(b_fix[0][:, :, 0:R, :], 0.0)

        # HAM warm-up: ~3.6us of dummy matmuls raise the PE clock gate to
        # 8/8 (2.4 GHz) while the first input DMA is in flight.
        for _ in range(9):
            nc.tensor.matmul(ps_dummy[:, 0:504], wz[:, 0:128], wz[:, 0:504],
                             start=True, stop=True)

        # ---- input loads: merged, front-loaded, on the gpsimd queue ----
        # Classes 0-2 have distinct buffers (3 b rotations, 3 a pool slots)
        # so their loads are all issued up-front; class 0 is split so the
        # first tile-row's dependencies (a tiles 0..7, b interior rows
        # 0..25) land first.  Class 3 reuses class 0's buffers, so its
        # loads MUST be issued after class 0's matmuls exist in program
        # order (dependency tracking is program-order based: a write
        # issued before its buffer's readers gets no WAR edge and
        # clobbers live data).  They are issued at the top of class 1's
        # body below.
        a_t = []
        b_t = []
        for cls in range(2):
            at = a_pool.tile([128, KCH, NTILE, 128], mm_dt, tag="a")
            bt = b_fix[cls % 3]
            a_t.append(at)
            b_t.append(bt)
            if cls == 0:
                nc.gpsimd.dma_start(
                    out=bt[:, :, R:R + 26, :],
                    in_=b_ap[cls, :, :, 0:26])
                nc.gpsimd.dma_start(
                    out=at[:, :, 0:NTX, :],
                    in_=a_ap[cls, :, :, 0:NTX])
                nc.gpsimd.dma_start(
                    out=bt[:, :, R + 26:R + HC, :],
                    in_=b_ap[cls, :, :, 26:HC])
                nc.gpsimd.dma_start(
                    out=at[:, :, NTX:NTILE, :],
                    in_=a_ap[cls, :, :, NTX:NTILE])
                nc.gpsimd.memset(b_fix[0][:, :, R + HC:H2P, :], 0.0)
            else:
                nc.gpsimd.dma_start(
                    out=bt[:, :, R:R + HC, :],
                    in_=b_ap[cls])
                nc.gpsimd.dma_start(out=at, in_=a_ap[cls])
                nc.gpsimd.memset(b_fix[cls][:, :, 0:R, :], 0.0)
                nc.gpsimd.memset(b_fix[cls][:, :, R + HC:H2P, :], 0.0)
        # class 2's b has its own rotation buffer -> load up-front too.
        b_t.append(b_fix[2])
        nc.gpsimd.dma_start(out=b_fix[2][:, :, R:R + HC, :], in_=b_ap[2])
        nc.gpsimd.memset(b_fix[2][:, :, 0:R, :], 0.0)
        nc.gpsimd.memset(b_fix[2][:, :, R + HC:H2P, :], 0.0)

        # single-wait "touchers": first PE consumer of each loaded region
        # carries exactly one DMA wait (fused LDW+MM supports only one).
        def touch(apx):
            nc.tensor.matmul(ps_dummy[0:1, 0:1], apx, apx, start=True,
                             stop=True)

        for cls in range(4):
            if cls == 1:
                # buffer-reusing loads must be issued after their buffer's
                # previous readers exist in program order (dependency
                # tracking is program-order based: a write issued before
                # its buffer's readers gets no WAR edge and clobbers live
                # data).  After class 0's matmuls: class 2's a (pool slot
                # 0) and class 3's b (rotation 0).
                at2 = a_pool.tile([128, KCH, NTILE, 128], mm_dt, tag="a")
                a_t.append(at2)
                nc.gpsimd.dma_start(out=at2, in_=a_ap[2])
                bt3 = b_fix[0]
                b_t.append(bt3)
                nc.gpsimd.dma_start(
                    out=bt3[:, :, R:R + HC, :],
                    in_=b_ap[3])
            if cls == 2:
                # after class 1's matmuls: class 3's a (pool slot 1).
                at3 = a_pool.tile([128, KCH, NTILE, 128], mm_dt, tag="a")
                a_t.append(at3)
                nc.gpsimd.dma_start(out=at3, in_=a_ap[3])
            at, bt = a_t[cls], b_t[cls]
            if cls == 0:
                # interleaved touchers: ty-row 0 only needs a tiles 0..7
                # and b interior rows 0..25, so the PE starts as soon as
                # those land; the rest streams in behind ty-row 0's
                # matmuls.
                touch(bt[:, 0, R, 0:1])
                touch(at[:, 0, 0, 0:1])
            else:
                touch(bt[:, 0, R, 0:1])
                touch(at[:, 0, 0, 0:1])

            for ty in range(NTY):
                if cls == 0 and ty == 1:
                    touch(bt[:, 0, R + 26, 0:1])
                    touch(at[:, 0, NTX, 0:1])
                ya = ty * H1T
                sb = s_pool.tile([128, NTX, NWIN], mybir.dt.bfloat16,
                                 tag="sb")
                for tx in range(NTX):
                    xa = tx * W1T
                    t = ty * NTX + tx
                    tg = cls * NTILE + t
                    ps = p_pool.tile([128, 2, 512], mybir.dt.float32)
                    # kc-outer: consecutive matmuls share the stationary
                    # operand; each 504-col write stays in one PSUM bank.
                    for kc in range(KCH):
                        lhsT = at[:, kc, t, :]
                        for h in range(2):
                            nc.tensor.matmul(
                                ps[:, h, 0:504],
                                lhsT,
                                bt[:, kc, ya + 18 * h:ya + 18 * (h + 1),
                                   xa:xa + W2T],
                                start=(kc == 0), stop=(kc == KCH - 1))
                    # one strided copy evacuates both PSUM banks; engines
                    # alternate per tile so each runs every other tile.
                    eng = nc.vector if (tg % 2 == 0) else nc.scalar
                    if cls == 3 and t == NTILE - 1:
                        # split the very last copy across both engines to
                        # shorten the kernel tail.
                        nc.vector.tensor_copy(sb[:, tx, 0:504], ps[:, 0, 0:504])
                        nc.scalar.copy(sb[:, tx, 504:NWIN], ps[:, 1, 0:504])
                    elif eng is nc.vector:
                        nc.vector.tensor_copy(sb[:, tx, :], ps[:, :, 0:504])
                    else:
                        nc.scalar.copy(sb[:, tx, :], ps[:, :, 0:504])
                # row-trimmed dump per ty-row, split by pixel u-quarter:
                # partitions with u in [4q, 4q+4) need window rows
                # [4q, 4q+24) only (672 of 1008 elements, -33% write
                # traffic).  32-partition runs of 1344B over 8 tiles.
                t0, t1 = ty * NTX, (ty + 1) * NTX
                if cls == 3 and ty == NTY - 1:
                    # last ty-row: dump in half-rows so the first half
                    # overlaps the last tiles' matmuls, and spread the
                    # second half's issues over the now-idle queues (a
                    # single queue serializes at ~0.7us per dma_start,
                    # which would all land after the final copy).
                    q_eng = [[nc.sync] * 4,
                             [nc.gpsimd, nc.scalar, nc.gpsimd, nc.scalar]]
                    for half in range(2):
                        ht0 = half * (NTX // 2)
                        for q in range(4):
                            q_eng[half][q].dma_start(
                                out=o_ap[cls, 32 * q:32 * (q + 1),
                                         t0 + ht0:t0 + ht0 + NTX // 2, :],
                                in_=sb[32 * q:32 * (q + 1),
                                       ht0:ht0 + NTX // 2,
                                       4 * q * W2T:4 * q * W2T + NTRIM])
                else:
                    for q in range(4):
                        nc.sync.dma_start(
                            out=o_ap[cls, 32 * q:32 * (q + 1), t0:t1, :],
                            in_=sb[32 * q:32 * (q + 1), :,
                                   4 * q * W2T:4 * q * W2T + NTRIM])
    nc.compile()
    return nc


def _host_prep(input1, input2):
    """Build device input arrays: parity classes, pad, fold in 1/C, cast."""
    x1 = (np.asarray(input1, dtype=np.float32) * np.float32(1.0 / C))
    # [B, C, H, W] -> [B, 4, C, HC, WC] with class = (y%2)*2 + (x%2)
    x1 = x1.reshape(B, C, HC, 2, WC, 2).transpose(0, 3, 5, 1, 2, 4)
    x1 = np.ascontiguousarray(x1).reshape(B, 4, C, HC, WC)
    # pre-tile: [.., HC, WC] -> [.., NTILE, 128] with pixel (u, v) contiguous
    x1 = x1.reshape(B, 4, C, NTY, H1T, NTX, W1T).transpose(0, 1, 2, 3, 5, 4, 6)
    x1 = np.ascontiguousarray(x1).reshape(B, 4, C, NTILE, 128)
    # channel c = kc*128 + p -> [B, 4, p(128), kc, NTILE, 128]
    x1 = x1.reshape(B, 4, KCH, 128, NTILE, 128).transpose(0, 1, 3, 2, 4, 5)
    x1 = np.ascontiguousarray(x1).astype(MM_NP)
    x2 = np.asarray(input2, dtype=np.float32)
    x2 = x2.reshape(B, C, HC, 2, WC, 2).transpose(0, 3, 5, 1, 2, 4)
    x2 = np.ascontiguousarray(x2).reshape(B, 4, C, HC, WC)
    # pad x only; y-pad rows live as persistent zeroed SBUF borders
    x2p = np.zeros((B, 4, C, HC, W2P), dtype=np.float32)
    x2p[:, :, :, :, R:R + WC] = x2
    x2p = x2p.reshape(B, 4, KCH, 128, HC, W2P).transpose(0, 1, 3, 2, 4, 5)
    x2p = np.ascontiguousarray(x2p).astype(MM_NP)
    return x1, x2p


def _in_maps(prepped):
    x1, x2p = prepped
    return [{"a": x1[b], "b": x2p[b]} for b in range(B)]


def _host_extract(res_o):
    """res_o: [4, 128, NTILE, NTRIM] row-trimmed window dump for one
    sample -> out [441, 96, 128] fp32.  Partition m = (uq, ul, v) holds
    window rows [4*uq, 4*uq + 24) so the stored row index is ul + i2."""
    r = np.ascontiguousarray(res_o).astype(np.float32).reshape(
        4, 4, H1T // 4, W1T, NTY, NTX, HTRIM, W2T)
    se = r.strides
    # V[cls, uq, ul, v, ty, tx, i2, j2] = r[cls, uq, ul, v, ty, tx,
    #                                       ul + i2, v + j2]
    V = np.lib.stride_tricks.as_strided(
        r,
        shape=(4, 4, H1T // 4, W1T, NTY, NTX, GW, GW),
        strides=(se[0], se[1], se[2] + se[6], se[3] + se[7],
                 se[4], se[5], se[6], se[7]),
    )
    # cls = (py, px); out[(i2,j2), (ty,uq,ul,py), (tx,v,px)]
    V = V.reshape(2, 2, 4, H1T // 4, W1T, NTY, NTX, GW, GW)
    out = V.transpose(7, 8, 5, 2, 3, 0, 6, 4, 1)
    return np.ascontiguousarray(out).reshape(NOUT, H, W)


def _make_runner(nc, n_cores=B):
    """Cached jitted SPMD runner (mirrors bass2jax.run_bass_via_pjrt, but
    reusable across calls so the NEFF compiles once per process)."""
    import jax
    from jax.sharding import Mesh, PartitionSpec
    from jax.experimental.shard_map import shard_map

    bass2jax.install_neuronx_cc_hook()

    partition_name = (nc.partition_id_tensor.name
                      if nc.partition_id_tensor else None)
    in_names, out_names, out_avals, zero_outs = [], [], [], []
    for alloc in nc.m.functions[0].allocations:
        if not isinstance(alloc, mybir.MemoryLocationSet):
            continue
        name = alloc.memorylocations[0].name
        if alloc.kind == "ExternalInput":
            if name != partition_name:
                in_names.append(name)
        elif alloc.kind == "ExternalOutput":
            out_names.append(name)
            shape = tuple(alloc.tensor_shape)
            dtype = mybir.dt.np(alloc.dtype)
            out_avals.append(jax.core.ShapedArray(shape, dtype))
            zero_outs.append(np.zeros(shape, dtype))
    n_params = len(in_names)
    n_outs = len(out_avals)
    all_names = in_names + out_names
    if partition_name is not None:
        all_names = all_names + [partition_name]
    donate = tuple(range(n_params, n_params + n_outs))

    def _body(*args):
        operands = list(args)
        if partition_name is not None:
            operands.append(bass2jax.partition_id_tensor())
        outs = bass2jax._bass_exec_p.bind(
            *operands,
            out_avals=tuple(out_avals),
            in_names=tuple(all_names),
            out_names=tuple(out_names),
            lowering_input_output_aliases=(),
            sim_require_finite=True,
            sim_require_nnan=True,
            nc=nc,
        )
        return tuple(outs)

    devices = jax.devices()[:n_cores]
    mesh = Mesh(np.asarray(devices), ("core",))
    in_specs = (PartitionSpec("core"),) * (n_params + n_outs)
    out_specs = (PartitionSpec("core"),) * n_outs
    sharded = jax.jit(
        shard_map(_body, mesh=mesh, in_specs=in_specs, out_specs=out_specs,
                  check_rep=False),
        donate_argnums=donate, keep_unused=True,
    )
    return {
        "fn": sharded, "in_names": in_names, "out_names": out_names,
        "out_avals": out_avals, "zero_outs": zero_outs, "mesh": mesh,
        "n_cores": n_cores,
    }


def _run_spmd(runner, in_maps):
    """Execute; returns list per core of {name: np.ndarray}."""
    import jax
    n_cores = runner["n_cores"]
    concat_in = [
        np.concatenate([np.asarray(in_maps[c][name]) for c in range(n_cores)], axis=0)
        for name in runner["in_names"]
    ]
    concat_zeros = [
        np.zeros((n_cores * z.shape[0], *z.shape[1:]), z.dtype)
        for z in runner["zero_outs"]
    ]
    out_arrs = runner["fn"](*concat_in, *concat_zeros)
    out_arrs = jax.block_until_ready(out_arrs)
    results = [
        {
            name: np.asarray(out_arrs[i]).reshape(n_cores, *runner["out_avals"][i].shape)[c]
            for i, name in enumerate(runner["out_names"])
        }
        for c in range(n_cores)
    ]
    return results


def time_exec(runner, in_maps, iters=3):
    """Device-execute wall time with inputs pre-transferred (seconds, min)."""
    import time as _time
    import jax
    from jax.sharding import NamedSharding, PartitionSpec
    n_cores = runner["n_cores"]
    sh = NamedSharding(runner["mesh"], PartitionSpec("core"))
    concat_in = [
        jax.device_put(
            np.concatenate([np.asarray(in_maps[c][name]) for c in range(n_cores)],
                           axis=0), sh)
        for name in runner["in_names"]
    ]
    best = None
    for _ in range(iters):
        zeros = [
            jax.device_put(
                np.zeros((n_cores * z.shape[0], *z.shape[1:]), z.dtype), sh)
            for z in runner["zero_outs"]
        ]
        jax.block_until_ready(zeros)
        jax.block_until_ready(concat_in)
        t0 = _time.perf_counter()
        outs = runner["fn"](*concat_in, *zeros)
        jax.block_until_ready(outs)
        dt = _time.perf_counter() - t0
        best = dt if best is None else min(best, dt)
    return best


def get_runner():
    if "runner" not in _CACHE:
        _CACHE["nc"] = _build_nc(MM_DT)
        _CACHE["runner"] = _make_runner(_CACHE["nc"])
    return _CACHE["runner"]


def kernel(input1, input2):
    assert input1.shape == (B, C, H, W) and input2.shape == (B, C, H, W)
    prepped = _host_prep(input1, input2)
    runner = get_runner()
    in_maps = _in_maps(prepped)
    results = _run_spmd(runner, in_maps)
    out = np.empty((B, NOUT, H, W), dtype=np.float32)
    for b in range(B):
        out[b] = _host_extract(results[b]["o"])
    return out
